# revision 15
# baseline (speedup 1.0000x reference)
"""Trainium2 Bass kernel for nn_FCGF_point_att_k (ragged segment attention pooling).

Math (per segment b of n=16384 points, full N=262144, C=32, F1=256, F2=1024):
    h   = relu(bn1(x @ w1.T + b1))                 # [n, 256]
    att = relu(bn2(h @ w2.T + b2))                 # [n, 1024]
    soft = softmax_over_points(att)                # per channel k
    res[b] = sum_p mean_k(soft[p,k]) * x[p] / n    -> l2-normalize rows

Key reductions used by this kernel:
  * BN folds into the GEMM weights/bias on the host (eval mode).
  * softmax max-subtraction is unnecessary for this value range; with
    e[p,k] = max(exp(z[p,k]), exp(-c2[k]))  (z = h @ W2fold, c2 = folded
    bias), e equals exp(max(z,-c2)) = exp(-c2[k])*exp(relu(z+c2)) whose
    per-channel scale cancels in M[c,k]/den[k].
  * the whole output reduces to G = [x | 1].T @ e per segment ([33, 1024]):
    rows 0..31 = M, row 32 = den; res[c] = sum_k M[c,k]/den[k] (host, tiny).

Engine balance (measured on the f32r predecessor: PE 91%, ACT 84%, DVE 72%):
  * L2 GEMM (the 17 GFLOP/core dominator) runs fp8e4 DoubleRow: K=256 in
    one pass, ~2x PE throughput. h is scaled x8 (folded into w1), w2 x16;
    exp un-scales via its free scale arg. End-to-end rel err ~1.2e-2.
  * e and the G path are bf16: DVE's clamp (vs exp(-c2)) runs 2x_1P, and
    exp consumes a whole [128,1024] 2-bank PSUM tile per ACTIVATE to
    amortize the ~310-cycle per-instruction overhead.
  * relu runs on DVE (tensor_scalar_max -> fp8 ht) to keep ScalarE
    exp-only.

Sharding: data-parallel, 2 whole segments per core on 8 cores; weights
replicated; per-core result is G [2, 33, 1024]; host combines.
"""

import numpy as np

# Problem shape (hardcoded per harness contract)
N, C_IN, F1, F2, B = 262144, 32, 256, 1024, 16
SEG = 16384
NCORES = 8
SEG_PER_CORE = B // NCORES          # 2
PTS = SEG_PER_CORE * SEG            # 32768 points per core
CH_PER_SEG = SEG // 128             # 128 chunks of 128 points per segment
NCHUNK = PTS // 128                 # 256
BLK = 512                           # L1 block (points)
XT_TILE = 2048                      # streamed xT tile width (points)
EPS_BN = 1e-5
EPS_NORM = 1e-12
SH = 8.0                            # h pre-scale (folded into w1a)
SW = 16.0                           # w2 pre-scale; exp scale = 1/(SH*SW)

_NC_CACHE = {}


def _build():
    """Build + compile the per-core Bass program."""
    import concourse.bacc as bacc
    import concourse.mybir as mybir
    import concourse.tile as tile

    F32 = mybir.dt.float32
    BF16 = mybir.dt.bfloat16
    FP8 = mybir.dt.float8e4
    ACT = mybir.ActivationFunctionType
    DR = mybir.MatmulPerfMode.DoubleRow

    nc = bacc.Bacc("TRN2", target_bir_lowering=False, debug=False)
    # xa is pre-transposed on the host to partition-major [128, NCHUNK*33]
    # so the DMA is contiguous per partition.
    d_xa = nc.dram_tensor("xa", [128, NCHUNK, 33], BF16, kind="ExternalInput")
    d_xt = nc.dram_tensor("xt", [SEG_PER_CORE, 33, SEG], BF16, kind="ExternalInput")
    d_w1 = nc.dram_tensor("w1a", [33, 2, 128], BF16, kind="ExternalInput")
    d_w2 = nc.dram_tensor("w2f", [128, 2, F2], FP8, kind="ExternalInput")
    d_qe = nc.dram_tensor("qexp", [128, F2], BF16, kind="ExternalInput")
    d_out = nc.dram_tensor("gout", [SEG_PER_CORE, 97, 512], F32, kind="ExternalOutput")

    with tile.TileContext(nc) as tc:
        with (
            tc.tile_pool(name="consts", bufs=1) as consts,
            tc.tile_pool(name="xtp", bufs=3) as xtp,
            tc.tile_pool(name="hp", bufs=3) as hp,
            tc.tile_pool(name="ep", bufs=4) as ep,
            tc.tile_pool(name="gop", bufs=2) as gop,
            tc.tile_pool(name="l1ps", bufs=2, space="PSUM") as l1ps,
            tc.tile_pool(name="attps", bufs=2, space="PSUM") as attps,
            tc.tile_pool(name="gps", bufs=1, space="PSUM") as gps,
        ):
            # DMA issue order follows per-queue FIFO, so order by deadline:
            # w1a + xt tile 0 gate the very first L1 matmul; w2f gates the
            # first L2; qexp gates the first DVE max; xa pieces gate the G
            # matmuls progressively and go last.
            w1a = consts.tile([33, 2, 128], BF16)
            nc.sync.dma_start(out=w1a, in_=d_w1.ap())
            xt0 = xtp.tile([33, XT_TILE], BF16, tag="xt")
            nc.sync.dma_start(out=xt0, in_=d_xt.ap()[0, :, 0:XT_TILE])
            # PE warm-up: ~2.5us of result-free matmuls on w1a right after
            # its DMA lands, so the HAM clock gate reaches K=8/8 (2.4 GHz)
            # before the first real matmul instead of ~20us into the kernel.
            wu = l1ps.tile([128, BLK], F32, tag="l1")
            for _ in range(24):
                nc.tensor.matmul(
                    wu[:, 0:256], w1a[:, 0, :], w1a[:, :, :], start=True, stop=True
                )
            w2f = consts.tile([128, 2, F2], FP8)
            nc.gpsimd.dma_start(out=w2f, in_=d_w2.ap())
            qexp = consts.tile([128, F2], BF16)
            nc.gpsimd.dma_start(out=qexp, in_=d_qe.ap())
            xa_splits = [0, 8, 32, 64, 128, 192, NCHUNK]
            xa_tiles = []
            for q in range(len(xa_splits) - 1):
                lo, hi = xa_splits[q], xa_splits[q + 1]
                t = consts.tile([128, hi - lo, 33], BF16, tag=f"xa{q}")
                eng = nc.sync if q % 2 == 0 else nc.gpsimd
                eng.dma_start(out=t, in_=d_xa.ap()[:, lo:hi, :])
                xa_tiles.append(t)

            def xa_chunk(gc):
                for q in range(len(xa_splits) - 1):
                    if gc < xa_splits[q + 1]:
                        return xa_tiles[q][:, gc - xa_splits[q], :]
                raise AssertionError

            for seg in range(SEG_PER_CORE):
                # gt packs both F2-halves in ONE PSUM bank via col-tiling:
                # kk=0 accumulates at partitions 0..32, kk=1 at 64..96
                # (tile_position=(0,64)); the two M=33 G matmuls then run
                # concurrently in disjoint column groups of the PE array.
                gt = gps.tile([128, 512], F32)
                cur_xt = None
                pend_g = None  # software-pipeline G by one chunk so the PE
                # queue never stalls waiting for e (exp+clamp latency)

                def issue_g(pg):
                    c, gc, e = pg
                    for kk in range(2):
                        nc.tensor.matmul(
                            gt[64 * kk : 64 * kk + 33, :],
                            xa_chunk(gc),
                            e[:, kk * 512 : (kk + 1) * 512],
                            start=(c == 0),
                            stop=(c == CH_PER_SEG - 1),
                        )

                for blk in range(SEG // BLK):  # 32 blocks of 512 points
                    xti, off = divmod(blk, XT_TILE // BLK)
                    if off == 0:
                        if seg == 0 and xti == 0:
                            cur_xt = xt0
                        else:
                            cur_xt = xtp.tile([33, XT_TILE], BF16, tag="xt", name="xt_t")
                            nc.sync.dma_start(
                                out=cur_xt,
                                in_=d_xt.ap()[seg, :, xti * XT_TILE : (xti + 1) * XT_TILE],
                            )
                    ht = hp.tile([128, 2, BLK], FP8)
                    for f1c in range(2):
                        l1t = l1ps.tile([128, BLK], F32, tag="l1")
                        nc.tensor.matmul(
                            l1t[:, :],
                            w1a[:, f1c, :],
                            cur_xt[:, off * BLK : (off + 1) * BLK],
                            start=True,
                            stop=True,
                        )
                        # relu + fp8 cast on DVE keeps ScalarE exp-only
                        nc.vector.tensor_scalar_max(ht[:, f1c, :], l1t[:, :], 0.0)
                    for sub in range(BLK // 128):  # 4 chunks of 128 points
                        c = blk * (BLK // 128) + sub
                        gc = seg * CH_PER_SEG + c
                        att2 = attps.tile([128, F2], F32, tag="att")
                        for kk in range(2):
                            nc.tensor.matmul(
                                att2[:, kk * 512 : (kk + 1) * 512],
                                ht[:, :, sub * 128 : (sub + 1) * 128],
                                w2f[:, :, kk * 512 : (kk + 1) * 512],
                                start=True,
                                stop=True,
                                perf_mode=DR,
                            )
                        e = ep.tile([128, F2], BF16)
                        nc.scalar.activation(e, att2, ACT.Exp, scale=1.0 / (SH * SW))
                        nc.vector.tensor_max(e, e, qexp)
                        if pend_g is not None:
                            issue_g(pend_g)
                        pend_g = (c, gc, e)
                issue_g(pend_g)
                gsb = gop.tile([97, 512], F32)
                nc.vector.tensor_copy(gsb, gt[0:97, :])
                nc.sync.dma_start(out=d_out.ap()[seg], in_=gsb)

    nc.compile()
    return nc


def _get_nc():
    if "v2" not in _NC_CACHE:
        _NC_CACHE["v2"] = _build()
    return _NC_CACHE["v2"]


def _prep_inputs(x, w1, b1, g1, be1, m1, v1, w2, b2, g2, be2, m2, v2):
    """Fold BN into GEMM weights, build per-core device input maps.

    w1a carries SH*(W1|c1) so relu(l1) = SH*h fits fp8e4 well; w2f carries
    SW*W2.T so the DR matmul yields SH*SW*z; exp applies scale=1/(SH*SW).
    qexp = exp(-c2): e = max(exp(z), exp(-c2)) == exp(max(z, -c2)).
    """
    import ml_dtypes

    f32 = np.float32
    bf16 = ml_dtypes.bfloat16
    fp8 = ml_dtypes.float8_e4m3
    x = np.asarray(x, f32)
    s1 = np.asarray(g1, f32) / np.sqrt(np.asarray(v1, f32) + EPS_BN)
    c1 = np.asarray(b1, f32) * s1 + np.asarray(be1, f32) - np.asarray(m1, f32) * s1
    s2 = np.asarray(g2, f32) / np.sqrt(np.asarray(v2, f32) + EPS_BN)
    c2 = np.asarray(b2, f32) * s2 + np.asarray(be2, f32) - np.asarray(m2, f32) * s2
    W1 = np.asarray(w1, f32) * s1[:, None]          # [256, 32]
    W2 = np.asarray(w2, f32) * s2[:, None]          # [1024, 256]

    w1a = np.empty((33, 2, 128), f32)
    w1a[:32] = W1.T.reshape(32, 2, 128)
    w1a[32] = c1.reshape(2, 128)
    w1a = (w1a * SH).astype(bf16)
    w2f = np.ascontiguousarray(
        (W2.T * SW).reshape(2, 128, F2).transpose(1, 0, 2)
    ).astype(fp8)
    qexp = np.ascontiguousarray(np.broadcast_to(np.exp(-c2), (128, F2))).astype(bf16)

    in_maps = []
    for i in range(NCORES):
        xs = x[i * PTS : (i + 1) * PTS]
        xa = np.empty((PTS, 33), f32)
        xa[:, :32] = xs
        xa[:, 32] = 1.0
        xt = np.ascontiguousarray(
            xa.reshape(SEG_PER_CORE, SEG, 33).transpose(0, 2, 1)
        ).astype(bf16)  # [2, 33, 16384]
        in_maps.append(
            {
                "xa": np.ascontiguousarray(
                    xa.reshape(NCHUNK, 128, 33).transpose(1, 0, 2)
                ).astype(bf16),
                "xt": xt,
                "w1a": w1a,
                "w2f": w2f,
                "qexp": qexp,
            }
        )
    return in_maps


def _postprocess(results, length):
    f32 = np.float32
    Gp = np.stack([r["gout"] for r in results]).reshape(B, 97, 512)
    G = np.concatenate([Gp[:, 0:33, :], Gp[:, 64:97, :]], axis=2)  # [B, 33, 1024]
    M = G[:, :32, :]
    den = G[:, 32, :]
    res = (M / den[:, None, :]).sum(-1) / F2
    res = res / np.asarray(length, f32)[:, None]
    nrm = np.sqrt((res * res).sum(1, keepdims=True))
    return (res / np.maximum(nrm, EPS_NORM)).astype(f32)


def run_on_device(inputs, trace=False, **kwargs):
    """Run the device portion; returns BassKernelResults."""
    from concourse.bass_utils import run_bass_kernel_spmd

    if trace:
        try:
            import ntff_hook  # noqa: PLC0415  # available only in the dev dir

            ntff_hook.install()
        except ImportError:
            pass
    in_maps = _prep_inputs(
        inputs["x"], inputs["w1"], inputs["b1"], inputs["g1"], inputs["be1"],
        inputs["m1"], inputs["v1"], inputs["w2"], inputs["b2"], inputs["g2"],
        inputs["be2"], inputs["m2"], inputs["v2"],
    )
    nc = _get_nc()
    res = run_bass_kernel_spmd(
        nc, in_maps, core_ids=list(range(NCORES)), trace=trace, **kwargs
    )
    return res


def kernel(x, length, w1, b1, g1, be1, m1, v1, w2, b2, g2, be2, m2, v2):
    inputs = dict(
        x=x, length=length, w1=w1, b1=b1, g1=g1, be1=be1, m1=m1, v1=v1,
        w2=w2, b2=b2, g2=g2, be2=be2, m2=m2, v2=v2,
    )
    res = run_on_device(inputs, trace=False)
    return _postprocess(res.results, length)


# revision 17
# speedup vs baseline: 1.0182x; 1.0182x over previous
"""Trainium2 Bass kernel for nn_FCGF_point_att_k (ragged segment attention pooling).

Math (per segment b of n=16384 points, full N=262144, C=32, F1=256, F2=1024):
    h   = relu(bn1(x @ w1.T + b1))                 # [n, 256]
    att = relu(bn2(h @ w2.T + b2))                 # [n, 1024]
    soft = softmax_over_points(att)                # per channel k
    res[b] = sum_p mean_k(soft[p,k]) * x[p] / n    -> l2-normalize rows

Key reductions used by this kernel:
  * BN folds into the GEMM weights/bias on the host (eval mode).
  * softmax max-subtraction is unnecessary for this value range; with
    e[p,k] = max(exp(z[p,k]), exp(-c2[k]))  (z = h @ W2fold, c2 = folded
    bias), e equals exp(max(z,-c2)) = exp(-c2[k])*exp(relu(z+c2)) whose
    per-channel scale cancels in M[c,k]/den[k].
  * the whole output reduces to G = [x | 1].T @ e per segment ([33, 1024]):
    rows 0..31 = M, row 32 = den; res[c] = sum_k M[c,k]/den[k] (host, tiny).

Engine balance (measured on the f32r predecessor: PE 91%, ACT 84%, DVE 72%):
  * L2 GEMM (the 17 GFLOP/core dominator) runs fp8e4 DoubleRow: K=256 in
    one pass, ~2x PE throughput. h is scaled x8 (folded into w1), w2 x16;
    exp un-scales via its free scale arg. End-to-end rel err ~1.2e-2.
  * e and the G path are bf16: DVE's clamp (vs exp(-c2)) runs 2x_1P, and
    exp consumes a whole [128,1024] 2-bank PSUM tile per ACTIVATE to
    amortize the ~310-cycle per-instruction overhead.
  * relu runs on DVE (tensor_scalar_max -> fp8 ht) to keep ScalarE
    exp-only.

Sharding: data-parallel, 2 whole segments per core on 8 cores; weights
replicated; per-core result is G [2, 33, 1024]; host combines.
"""

import numpy as np

# Problem shape (hardcoded per harness contract)
N, C_IN, F1, F2, B = 262144, 32, 256, 1024, 16
SEG = 16384
NCORES = 8
SEG_PER_CORE = B // NCORES          # 2
PTS = SEG_PER_CORE * SEG            # 32768 points per core
CH_PER_SEG = SEG // 128             # 128 chunks of 128 points per segment
NCHUNK = PTS // 128                 # 256
BLK = 512                           # L1 block (points)
XT_TILE = 2048                      # streamed xT tile width (points)
EPS_BN = 1e-5
EPS_NORM = 1e-12
SH = 8.0                            # h pre-scale (folded into w1a)
SW = 16.0                           # w2 pre-scale; exp scale = 1/(SH*SW)

_NC_CACHE = {}


def _build():
    """Build + compile the per-core Bass program."""
    import concourse.bacc as bacc
    import concourse.mybir as mybir
    import concourse.tile as tile

    F32 = mybir.dt.float32
    BF16 = mybir.dt.bfloat16
    FP8 = mybir.dt.float8e4
    ACT = mybir.ActivationFunctionType
    DR = mybir.MatmulPerfMode.DoubleRow

    nc = bacc.Bacc("TRN2", target_bir_lowering=False, debug=False)
    # xa is pre-transposed on the host to partition-major [128, NCHUNK*33]
    # so the DMA is contiguous per partition.
    d_xa = nc.dram_tensor("xa", [128, NCHUNK, 33], BF16, kind="ExternalInput")
    d_xt = nc.dram_tensor("xt", [SEG_PER_CORE, 33, SEG], BF16, kind="ExternalInput")
    d_w1 = nc.dram_tensor("w1a", [33, 2, 128], BF16, kind="ExternalInput")
    d_w2 = nc.dram_tensor("w2f", [128, 2, F2], FP8, kind="ExternalInput")
    d_qe = nc.dram_tensor("qexp", [128, F2], BF16, kind="ExternalInput")
    d_out = nc.dram_tensor("gout", [SEG_PER_CORE, 97, 512], F32, kind="ExternalOutput")

    with tile.TileContext(nc) as tc:
        with (
            tc.tile_pool(name="consts", bufs=1) as consts,
            tc.tile_pool(name="xtp", bufs=3) as xtp,
            tc.tile_pool(name="hp", bufs=3) as hp,
            tc.tile_pool(name="ep", bufs=4) as ep,
            tc.tile_pool(name="gop", bufs=2) as gop,
            tc.tile_pool(name="l1ps", bufs=2, space="PSUM") as l1ps,
            tc.tile_pool(name="attps", bufs=2, space="PSUM") as attps,
            tc.tile_pool(name="gps", bufs=1, space="PSUM") as gps,
        ):
            # DMA issue order follows per-queue FIFO, so order by deadline:
            # w1a + xt tile 0 gate the very first L1 matmul; w2f gates the
            # first L2; qexp gates the first DVE max; xa pieces gate the G
            # matmuls progressively and go last.
            w1a = consts.tile([33, 2, 128], BF16)
            nc.sync.dma_start(out=w1a, in_=d_w1.ap())
            xt0 = xtp.tile([33, XT_TILE], BF16, tag="xt")
            nc.sync.dma_start(out=xt0, in_=d_xt.ap()[0, :, 0:XT_TILE])
            # PE warm-up: ~2.5us of result-free matmuls on w1a right after
            # its DMA lands, so the HAM clock gate reaches K=8/8 (2.4 GHz)
            # before the first real matmul instead of ~20us into the kernel.
            wu = l1ps.tile([128, BLK], F32, tag="l1")
            for _ in range(24):
                nc.tensor.matmul(
                    wu[:, 0:256], w1a[:, 0, :], w1a[:, :, :], start=True, stop=True
                )
            w2f = consts.tile([128, 2, F2], FP8)
            nc.gpsimd.dma_start(out=w2f, in_=d_w2.ap())
            qexp = consts.tile([128, F2], BF16)
            nc.gpsimd.dma_start(out=qexp, in_=d_qe.ap())
            xa_splits = [0, 8, 32, 64, 128, 192, NCHUNK]
            xa_tiles = []
            for q in range(len(xa_splits) - 1):
                lo, hi = xa_splits[q], xa_splits[q + 1]
                t = consts.tile([128, hi - lo, 33], BF16, tag=f"xa{q}")
                eng = nc.sync if q % 2 == 0 else nc.gpsimd
                eng.dma_start(out=t, in_=d_xa.ap()[:, lo:hi, :])
                xa_tiles.append(t)

            def xa_chunk(gc):
                for q in range(len(xa_splits) - 1):
                    if gc < xa_splits[q + 1]:
                        return xa_tiles[q][:, gc - xa_splits[q], :]
                raise AssertionError

            for seg in range(SEG_PER_CORE):
                # gt packs both F2-halves in ONE PSUM bank via col-tiling:
                # kk=0 accumulates at partitions 0..32, kk=1 at 64..96
                # (tile_position=(0,64)); the two M=33 G matmuls then run
                # concurrently in disjoint column groups of the PE array.
                gt = gps.tile([128, 512], F32)
                cur_xt = None
                for blk in range(SEG // BLK):  # 32 blocks of 512 points
                    xti, off = divmod(blk, XT_TILE // BLK)
                    if off == 0:
                        if seg == 0 and xti == 0:
                            cur_xt = xt0
                        else:
                            cur_xt = xtp.tile([33, XT_TILE], BF16, tag="xt", name="xt_t")
                            nc.sync.dma_start(
                                out=cur_xt,
                                in_=d_xt.ap()[seg, :, xti * XT_TILE : (xti + 1) * XT_TILE],
                            )
                    ht = hp.tile([128, 2, BLK], FP8)
                    for f1c in range(2):
                        l1t = l1ps.tile([128, BLK], F32, tag="l1")
                        nc.tensor.matmul(
                            l1t[:, :],
                            w1a[:, f1c, :],
                            cur_xt[:, off * BLK : (off + 1) * BLK],
                            start=True,
                            stop=True,
                        )
                        # relu + fp8 cast on DVE keeps ScalarE exp-only
                        nc.vector.tensor_scalar_max(ht[:, f1c, :], l1t[:, :], 0.0)
                    for sub in range(BLK // 128):  # 4 chunks of 128 points
                        c = blk * (BLK // 128) + sub
                        gc = seg * CH_PER_SEG + c
                        att2 = attps.tile([128, F2], F32, tag="att")
                        for kk in range(2):
                            nc.tensor.matmul(
                                att2[:, kk * 512 : (kk + 1) * 512],
                                ht[:, :, sub * 128 : (sub + 1) * 128],
                                w2f[:, :, kk * 512 : (kk + 1) * 512],
                                start=True,
                                stop=True,
                                perf_mode=DR,
                            )
                        e = ep.tile([128, F2], BF16)
                        nc.scalar.activation(e, att2, ACT.Exp, scale=1.0 / (SH * SW))
                        nc.vector.tensor_max(e, e, qexp)
                        for kk in range(2):
                            nc.tensor.matmul(
                                gt[64 * kk : 64 * kk + 33, :],
                                xa_chunk(gc),
                                e[:, kk * 512 : (kk + 1) * 512],
                                start=(c == 0),
                                stop=(c == CH_PER_SEG - 1),
                            )
                gsb = gop.tile([97, 512], F32)
                nc.vector.tensor_copy(gsb, gt[0:97, :])
                nc.sync.dma_start(out=d_out.ap()[seg], in_=gsb)

    nc.compile()
    return nc


def _get_nc():
    if "v2" not in _NC_CACHE:
        _NC_CACHE["v2"] = _build()
    return _NC_CACHE["v2"]


def _prep_inputs(x, w1, b1, g1, be1, m1, v1, w2, b2, g2, be2, m2, v2):
    """Fold BN into GEMM weights, build per-core device input maps.

    w1a carries SH*(W1|c1) so relu(l1) = SH*h fits fp8e4 well; w2f carries
    SW*W2.T so the DR matmul yields SH*SW*z; exp applies scale=1/(SH*SW).
    qexp = exp(-c2): e = max(exp(z), exp(-c2)) == exp(max(z, -c2)).
    """
    import ml_dtypes

    f32 = np.float32
    bf16 = ml_dtypes.bfloat16
    fp8 = ml_dtypes.float8_e4m3
    x = np.asarray(x, f32)
    s1 = np.asarray(g1, f32) / np.sqrt(np.asarray(v1, f32) + EPS_BN)
    c1 = np.asarray(b1, f32) * s1 + np.asarray(be1, f32) - np.asarray(m1, f32) * s1
    s2 = np.asarray(g2, f32) / np.sqrt(np.asarray(v2, f32) + EPS_BN)
    c2 = np.asarray(b2, f32) * s2 + np.asarray(be2, f32) - np.asarray(m2, f32) * s2
    W1 = np.asarray(w1, f32) * s1[:, None]          # [256, 32]
    W2 = np.asarray(w2, f32) * s2[:, None]          # [1024, 256]

    w1a = np.empty((33, 2, 128), f32)
    w1a[:32] = W1.T.reshape(32, 2, 128)
    w1a[32] = c1.reshape(2, 128)
    w1a = (w1a * SH).astype(bf16)
    w2f = np.ascontiguousarray(
        (W2.T * SW).reshape(2, 128, F2).transpose(1, 0, 2)
    ).astype(fp8)
    qexp = np.ascontiguousarray(np.broadcast_to(np.exp(-c2), (128, F2))).astype(bf16)

    in_maps = []
    for i in range(NCORES):
        xs = x[i * PTS : (i + 1) * PTS]
        xa = np.empty((PTS, 33), f32)
        xa[:, :32] = xs
        xa[:, 32] = 1.0
        xt = np.ascontiguousarray(
            xa.reshape(SEG_PER_CORE, SEG, 33).transpose(0, 2, 1)
        ).astype(bf16)  # [2, 33, 16384]
        in_maps.append(
            {
                "xa": np.ascontiguousarray(
                    xa.reshape(NCHUNK, 128, 33).transpose(1, 0, 2)
                ).astype(bf16),
                "xt": xt,
                "w1a": w1a,
                "w2f": w2f,
                "qexp": qexp,
            }
        )
    return in_maps


def _postprocess(results, length):
    f32 = np.float32
    Gp = np.stack([r["gout"] for r in results]).reshape(B, 97, 512)
    G = np.concatenate([Gp[:, 0:33, :], Gp[:, 64:97, :]], axis=2)  # [B, 33, 1024]
    M = G[:, :32, :]
    den = G[:, 32, :]
    res = (M / den[:, None, :]).sum(-1) / F2
    res = res / np.asarray(length, f32)[:, None]
    nrm = np.sqrt((res * res).sum(1, keepdims=True))
    return (res / np.maximum(nrm, EPS_NORM)).astype(f32)


def run_on_device(inputs, trace=False, **kwargs):
    """Run the device portion; returns BassKernelResults."""
    from concourse.bass_utils import run_bass_kernel_spmd

    if trace:
        try:
            import ntff_hook  # noqa: PLC0415  # available only in the dev dir

            ntff_hook.install()
        except ImportError:
            pass
    in_maps = _prep_inputs(
        inputs["x"], inputs["w1"], inputs["b1"], inputs["g1"], inputs["be1"],
        inputs["m1"], inputs["v1"], inputs["w2"], inputs["b2"], inputs["g2"],
        inputs["be2"], inputs["m2"], inputs["v2"],
    )
    nc = _get_nc()
    res = run_bass_kernel_spmd(
        nc, in_maps, core_ids=list(range(NCORES)), trace=trace, **kwargs
    )
    return res


def kernel(x, length, w1, b1, g1, be1, m1, v1, w2, b2, g2, be2, m2, v2):
    inputs = dict(
        x=x, length=length, w1=w1, b1=b1, g1=g1, be1=be1, m1=m1, v1=v1,
        w2=w2, b2=b2, g2=g2, be2=be2, m2=m2, v2=v2,
    )
    res = run_on_device(inputs, trace=False)
    return _postprocess(res.results, length)
